# revision 5
# baseline (speedup 1.0000x reference)
"""CantorExpert MoE-routing kernel for 8x TRN2 NeuronCores.

Strategy (data-parallel, per sharding hint):
  - Host: fingerprint-region routing (mask -> gather indices), gather the
    expert's feature slice my = tokens[:, idx, 1024:1536], transpose to
    feature-major layout, pad tokens to a multiple of 8*512, shard tokens
    across the 8 cores. Small weights are replicated. The pentachoron
    projections are folded into fused [512, 15] weights (dirs @ W).T so the
    device never needs Q^T. All device inputs are pre-swizzled on host into
    [128, *] partition-major contiguous blocks so every load is one cheap
    fully-contiguous HWDGE DMA.
  - Device (per core, T tokens): gate MLP via matmuls + GELU/Sigmoid LUTs,
    producing a per-token gate g = aw*sigmoid(.)+(1-aw) in token-partition
    column layout; QKV + fused penta projections as float32r matmuls
    (1 cycle/row, ~1.6e-4 rel err); the token gating is folded into the
    PSUM->SBUF eviction as a per-partition scale (free).
  - Host: concatenate per-core outputs, strip padding, reshape.
"""

import numpy as np

# Config constants (CantorExpertConfig, expert 2 of 8)
EXPERT_ID = 2
NUM_EXPERTS = 8
FULL = 4096
EDIM = 1024
SLICE = FULL // NUM_EXPERTS  # 512
OVERLAP = 0.5
_base = 1.0 / NUM_EXPERTS
_ext = _base * OVERLAP
FP_MIN = max(0.0, EXPERT_ID / NUM_EXPERTS - _ext)  # 0.1875
FP_MAX = min(1.0, (EXPERT_ID + 1) / NUM_EXPERTS + _ext)  # 0.4375
SL_START = EXPERT_ID * SLICE  # 1024
SL_END = SL_START + SLICE  # 1536

N_CORES = 8
TCHUNK = 512  # token chunk (= max fp32 matmul free dim = 1 PSUM bank)
PCOLS = 16  # fused penta weight columns (15 used, padded for fp32r dst rules)
KC = SLICE // 128  # 4 contraction chunks
NB = EDIM // TCHUNK  # 2 n-blocks per projection


def _swz(a):
    """[512, N] feature-major -> [128, KC*N] partition-major contiguous."""
    n = a.shape[1]
    return np.ascontiguousarray(
        a.reshape(KC, 128, n).transpose(1, 0, 2).reshape(128, KC * n))


def _trace_kernel(nc, tc, tile, mybir, T, aw, b2):
    """Build the per-core program. T = tokens per core (multiple of 512)."""
    f32 = mybir.dt.float32
    f32r = mybir.dt.float32r
    GELU = mybir.ActivationFunctionType.Gelu
    SIGM = mybir.ActivationFunctionType.Sigmoid
    COPY = mybir.ActivationFunctionType.Copy

    NT = T // 128  # token tiles
    NG = T // TCHUNK  # gate chunks
    MPG = TCHUNK // 128  # token tiles per gate chunk

    myT = nc.dram_tensor("myT", [128, KC * T], f32r, kind="ExternalInput").ap()
    wg1 = nc.dram_tensor("wg1", [128, KC * 128], f32r, kind="ExternalInput").ap()
    bgw = nc.dram_tensor("bgw", [128, 2], f32, kind="ExternalInput").ap()
    wqkv = [
        nc.dram_tensor(f"w{n}{i}", [128, KC * TCHUNK], f32r,
                       kind="ExternalInput").ap()
        for n in "qkv" for i in range(NB)
    ]
    wp = nc.dram_tensor("wp", [128, KC * PCOLS], f32r, kind="ExternalInput").ap()
    q_out = nc.dram_tensor("q", [T, EDIM], f32, kind="ExternalOutput").ap()
    k_out = nc.dram_tensor("k", [T, EDIM], f32, kind="ExternalOutput").ap()
    v_out = nc.dram_tensor("v", [T, EDIM], f32, kind="ExternalOutput").ap()
    p_out = nc.dram_tensor("p", [T, PCOLS], f32, kind="ExternalOutput").ap()

    from contextlib import ExitStack

    with ExitStack() as ctx:
        singles = ctx.enter_context(tc.tile_pool(name="singles", bufs=1))
        pact = ctx.enter_context(tc.tile_pool(name="pact", bufs=2))
        ph = ctx.enter_context(tc.tile_pool(name="ph", bufs=1, space="PSUM"))
        pgl = ctx.enter_context(tc.tile_pool(name="pgl", bufs=2, space="PSUM"))
        pq = ctx.enter_context(tc.tile_pool(name="pq", bufs=3, space="PSUM"))
        pps = ctx.enter_context(tc.tile_pool(name="pps", bufs=2, space="PSUM"))
        po = ctx.enter_context(tc.tile_pool(name="po", bufs=3))

        # --- loads; order = critical path: gate needs mt+w1t, QKV needs wq ---
        mt = singles.tile([128, KC, T], f32r)
        nc.sync.dma_start(out=mt, in_=myT.rearrange("p (c t) -> p c t", c=KC))
        w1t = singles.tile([128, KC, 128], f32r)
        nc.sync.dma_start(out=w1t, in_=wg1.rearrange("p (c h) -> p c h", c=KC))
        bgwt = singles.tile([128, 2], f32)
        nc.sync.dma_start(out=bgwt, in_=bgw)
        wts = []
        for j in range(3 * NB):
            wt = singles.tile([128, KC, TCHUNK], f32r, tag=f"wt{j}")
            nc.sync.dma_start(out=wt,
                              in_=wqkv[j].rearrange("p (c n) -> p c n", c=KC))
            wts.append(wt)
        wpt = singles.tile([128, KC, PCOLS], f32r)
        nc.sync.dma_start(out=wpt, in_=wp.rearrange("p (c n) -> p c n", c=KC))

        bg1t = bgwt[:, 0:1]
        wg2t = bgwt[:, 1:2]

        # --- gate phase: g[t] = aw*sigmoid(MLP(my)[t]) + (1-aw) ---
        gcol_all = singles.tile([128, NT], f32)
        for g in range(NG):
            sl = slice(g * TCHUNK, (g + 1) * TCHUNK)
            hps = ph.tile([128, TCHUNK], f32, tag="hps")
            for c in range(KC):
                nc.tensor.matmul(
                    hps, lhsT=w1t[:, c, :], rhs=mt[:, c, sl],
                    start=(c == 0), stop=(c == KC - 1),
                )
            hact = pact.tile([128, TCHUNK], f32, tag="hact")
            nc.scalar.activation(out=hact, in_=hps, func=GELU, bias=bg1t)
            for mi in range(MPG):
                m = g * MPG + mi
                glps = pgl.tile([128, 1], f32, tag="glps")
                nc.tensor.matmul(
                    glps, lhsT=hact[:, mi * 128:(mi + 1) * 128], rhs=wg2t,
                    start=True, stop=True,
                )
                gc = gcol_all[:, m:m + 1]
                nc.scalar.activation(out=gc, in_=glps, func=SIGM, bias=b2)
                nc.scalar.activation(out=gc, in_=gc, func=COPY,
                                     bias=1.0 - aw, scale=aw)

        # --- fused penta projections first (tiny, frees the tail) ---
        op_all = singles.tile([128, NT, PCOLS], f32)
        for m in range(NT):
            tsl = slice(m * 128, (m + 1) * 128)
            psp = pps.tile([128, PCOLS], f32, tag="psp")
            for c in range(KC):
                nc.tensor.matmul(
                    psp, lhsT=mt[:, c, tsl], rhs=wpt[:, c, :],
                    start=(c == 0), stop=(c == KC - 1),
                )
            nc.vector.tensor_scalar_mul(out=op_all[:, m, :], in0=psp,
                                        scalar1=gcol_all[:, m:m + 1])
        nc.sync.dma_start(out=p_out.rearrange("(m p) j -> p m j", p=128),
                          in_=op_all)

        # --- QKV projections, gating folded into eviction scale ---
        ev = 0
        for m in range(NT):
            tsl = slice(m * 128, (m + 1) * 128)
            gc = gcol_all[:, m:m + 1]
            for pi, out_dram in enumerate((q_out, k_out, v_out)):
                o = po.tile([128, EDIM], f32, tag="o")
                for nb in range(NB):
                    nsl = slice(nb * TCHUNK, (nb + 1) * TCHUNK)
                    wt = wts[pi * NB + nb]
                    ps = pq.tile([128, TCHUNK], f32, tag="ps")
                    for c in range(KC):
                        nc.tensor.matmul(
                            ps, lhsT=mt[:, c, tsl], rhs=wt[:, c, :],
                            start=(c == 0), stop=(c == KC - 1),
                        )
                    if ev % 3 == 2:
                        nc.scalar.activation(out=o[:, nsl], in_=ps, func=COPY,
                                             bias=0.0, scale=gc)
                    else:
                        nc.vector.tensor_scalar_mul(out=o[:, nsl], in0=ps,
                                                    scalar1=gc)
                    ev += 1
                nc.sync.dma_start(out=out_dram[tsl, :], in_=o)


def _prep_inputs(tokens, fingerprints, W_g1, b_g1, W_g2, b_g2, alpha, Wq, Wk,
                 Wv, pentachoron):
    """Host-side routing, gather, padding, weight prep. Returns
    (in_maps, T, n_tok, B, Psel, aw, b2) or None if no token selected."""
    f32 = np.float32
    tokens = np.asarray(tokens)
    fingerprints = np.asarray(fingerprints)
    B = tokens.shape[0]

    mask = (fingerprints >= FP_MIN) & (fingerprints < FP_MAX)
    idx = np.nonzero(mask)[0]
    Psel = int(idx.shape[0])
    n_tok = B * Psel
    if n_tok == 0:
        return None

    my = tokens[:, idx, SL_START:SL_END].astype(f32)  # [B, Psel, 512]
    flat = my.reshape(n_tok, SLICE)
    T = -(-n_tok // (N_CORES * TCHUNK)) * TCHUNK
    flatT = np.zeros((SLICE, N_CORES * T), f32)
    flatT[:, :n_tok] = flat.T

    W_g1 = np.asarray(W_g1, f32)
    W_g2 = np.asarray(W_g2, f32)
    b_g1 = np.asarray(b_g1, f32)
    b_g2 = np.asarray(b_g2, f32)
    alpha32 = np.asarray(alpha, f32)
    Wq = np.asarray(Wq, f32)
    Wk = np.asarray(Wk, f32)
    Wv = np.asarray(Wv, f32)
    penta = np.asarray(pentachoron, f32)

    aw = float(1.0 / (1.0 + np.exp(-alpha32)))
    b2 = float(b_g2.reshape(-1)[0])
    dirs = penta / np.linalg.norm(penta, axis=-1, keepdims=True)  # [5, EDIM]
    wp = np.concatenate(
        [
            (W.T.astype(np.float64) @ dirs.T.astype(np.float64)).astype(f32)
            for W in (Wq, Wk, Wv)
        ],
        axis=1,
    )  # [512, 15]
    wp = np.concatenate([wp, np.zeros((SLICE, PCOLS - 15), f32)], axis=1)

    bgw = np.stack([b_g1, W_g2.reshape(-1)], axis=1)  # [128, 2]

    in_common = {
        "wg1": _swz(np.ascontiguousarray(W_g1.T)),
        "bgw": np.ascontiguousarray(bgw),
        "wp": _swz(wp),
    }
    for nm, W in (("q", Wq), ("k", Wk), ("v", Wv)):
        WT = np.ascontiguousarray(W.T)  # [512, 1024]
        for i in range(NB):
            in_common[f"w{nm}{i}"] = _swz(WT[:, i * TCHUNK:(i + 1) * TCHUNK])

    in_maps = [
        {"myT": _swz(flatT[:, c * T:(c + 1) * T]), **in_common}
        for c in range(N_CORES)
    ]
    return in_maps, T, n_tok, B, Psel, aw, b2


def _build(T, aw, b2):
    import concourse.mybir as mybir
    import concourse.tile as tile
    from concourse import bacc

    nc = bacc.Bacc("TRN2", target_bir_lowering=False, debug=False,
                   num_devices=N_CORES)
    with tile.TileContext(nc) as tc:
        _trace_kernel(nc, tc, tile, mybir, T, aw, b2)
    nc.compile()
    return nc


def _unshard(res, T, n_tok, B, Psel):
    q = np.concatenate([res.results[c]["q"] for c in range(N_CORES)], axis=0)
    k = np.concatenate([res.results[c]["k"] for c in range(N_CORES)], axis=0)
    v = np.concatenate([res.results[c]["v"] for c in range(N_CORES)], axis=0)
    p = np.concatenate([res.results[c]["p"] for c in range(N_CORES)], axis=0)

    Q = q[:n_tok].reshape(B, Psel, EDIM)
    K = k[:n_tok].reshape(B, Psel, EDIM)
    V = v[:n_tok].reshape(B, Psel, EDIM)
    p = p[:n_tok]  # [n_tok, PCOLS]
    Qp = np.ascontiguousarray(p[:, 0:5].T).reshape(5, B, Psel)
    Kp = np.ascontiguousarray(p[:, 5:10].T).reshape(5, B, Psel)
    Vp = np.ascontiguousarray(p[:, 10:15].T).reshape(5, B, Psel)
    return (Q, K, V, Qp, Kp, Vp)


def kernel(tokens, fingerprints, W_g1, b_g1, W_g2, b_g2, alpha, Wq, Wk, Wv,
           pentachoron):
    from concourse.bass_utils import run_bass_kernel_spmd

    prep = _prep_inputs(tokens, fingerprints, W_g1, b_g1, W_g2, b_g2, alpha,
                        Wq, Wk, Wv, pentachoron)
    if prep is None:
        B = np.asarray(tokens).shape[0]
        z = np.zeros((B, 0, EDIM), np.float32)
        zp = np.zeros((5, B, 0), np.float32)
        return (z, z.copy(), z.copy(), zp, zp.copy(), zp.copy())
    in_maps, T, n_tok, B, Psel, aw, b2 = prep

    nc = _build(T, aw, b2)
    res = run_bass_kernel_spmd(nc, in_maps, list(range(N_CORES)))
    return _unshard(res, T, n_tok, B, Psel)


# revision 6
# speedup vs baseline: 1.0320x; 1.0320x over previous
"""CantorExpert MoE-routing kernel for 8x TRN2 NeuronCores.

Strategy (data-parallel, per sharding hint):
  - Host: fingerprint-region routing (mask -> gather indices), gather the
    expert's feature slice my = tokens[:, idx, 1024:1536], transpose to
    feature-major layout, pad tokens to a multiple of 8*512, shard tokens
    across the 8 cores. Small weights are replicated. The pentachoron
    projections are folded into fused [512, 15] weights (dirs @ W).T so the
    device never needs Q^T. All device inputs are pre-swizzled on host into
    [128, *] partition-major contiguous blocks so every load is one cheap
    fully-contiguous HWDGE DMA.
  - Device (per core, T tokens): gate MLP via matmuls + GELU/Sigmoid LUTs,
    producing a per-token gate g = aw*sigmoid(.)+(1-aw) in token-partition
    column layout; QKV + fused penta projections as float32r matmuls
    (1 cycle/row, ~1.6e-4 rel err); the token gating is folded into the
    PSUM->SBUF eviction as a per-partition scale (free).
  - Host: concatenate per-core outputs, strip padding, reshape.
"""

import numpy as np

# Config constants (CantorExpertConfig, expert 2 of 8)
EXPERT_ID = 2
NUM_EXPERTS = 8
FULL = 4096
EDIM = 1024
SLICE = FULL // NUM_EXPERTS  # 512
OVERLAP = 0.5
_base = 1.0 / NUM_EXPERTS
_ext = _base * OVERLAP
FP_MIN = max(0.0, EXPERT_ID / NUM_EXPERTS - _ext)  # 0.1875
FP_MAX = min(1.0, (EXPERT_ID + 1) / NUM_EXPERTS + _ext)  # 0.4375
SL_START = EXPERT_ID * SLICE  # 1024
SL_END = SL_START + SLICE  # 1536

N_CORES = 8
TCHUNK = 512  # token chunk (= max fp32 matmul free dim = 1 PSUM bank)
PCOLS = 16  # fused penta weight columns (15 used, padded for fp32r dst rules)
KC = SLICE // 128  # 4 contraction chunks
NB = EDIM // TCHUNK  # 2 n-blocks per projection


def _swz(a):
    """[512, N] feature-major -> [128, KC*N] partition-major contiguous."""
    n = a.shape[1]
    return np.ascontiguousarray(
        a.reshape(KC, 128, n).transpose(1, 0, 2).reshape(128, KC * n))


def _trace_kernel(nc, tc, tile, mybir, T, aw, b2):
    """Build the per-core program. T = tokens per core (multiple of 512)."""
    f32 = mybir.dt.float32
    f32r = mybir.dt.float32r
    GELU = mybir.ActivationFunctionType.Gelu
    SIGM = mybir.ActivationFunctionType.Sigmoid
    COPY = mybir.ActivationFunctionType.Copy

    NT = T // 128  # token tiles
    NG = T // TCHUNK  # gate chunks
    MPG = TCHUNK // 128  # token tiles per gate chunk

    myT = nc.dram_tensor("myT", [128, KC * T], f32r, kind="ExternalInput").ap()
    wg1 = nc.dram_tensor("wg1", [128, KC * 128], f32r, kind="ExternalInput").ap()
    bgw = nc.dram_tensor("bgw", [128, 2], f32, kind="ExternalInput").ap()
    wqkv = [
        nc.dram_tensor(f"w{n}{i}", [128, KC * TCHUNK], f32r,
                       kind="ExternalInput").ap()
        for n in "qkv" for i in range(NB)
    ]
    wp = nc.dram_tensor("wp", [128, KC * PCOLS], f32r, kind="ExternalInput").ap()
    q_out = nc.dram_tensor("q", [T, EDIM], f32, kind="ExternalOutput").ap()
    k_out = nc.dram_tensor("k", [T, EDIM], f32, kind="ExternalOutput").ap()
    v_out = nc.dram_tensor("v", [T, EDIM], f32, kind="ExternalOutput").ap()
    p_out = nc.dram_tensor("p", [T, PCOLS], f32, kind="ExternalOutput").ap()

    from contextlib import ExitStack

    with ExitStack() as ctx:
        singles = ctx.enter_context(tc.tile_pool(name="singles", bufs=1))
        pact = ctx.enter_context(tc.tile_pool(name="pact", bufs=2))
        ph = ctx.enter_context(tc.tile_pool(name="ph", bufs=1, space="PSUM"))
        pgl = ctx.enter_context(tc.tile_pool(name="pgl", bufs=2, space="PSUM"))
        pq = ctx.enter_context(tc.tile_pool(name="pq", bufs=3, space="PSUM"))
        pps = ctx.enter_context(tc.tile_pool(name="pps", bufs=2, space="PSUM"))
        po = ctx.enter_context(tc.tile_pool(name="po", bufs=4))

        # --- loads; order = critical path: gate needs mt+w1t, QKV needs wq ---
        mts = []
        for c in range(KC):
            mtc = singles.tile([128, T], f32r, tag=f"mt{c}")
            nc.sync.dma_start(out=mtc, in_=myT[:, c * T:(c + 1) * T])
            mts.append(mtc)
        w1t = singles.tile([128, KC, 128], f32r)
        nc.sync.dma_start(out=w1t, in_=wg1.rearrange("p (c h) -> p c h", c=KC))
        bgwt = singles.tile([128, 2], f32)
        nc.sync.dma_start(out=bgwt, in_=bgw)
        wpt = singles.tile([128, KC, PCOLS], f32r)
        nc.sync.dma_start(out=wpt, in_=wp.rearrange("p (c n) -> p c n", c=KC))
        wts = []
        for j in range(3 * NB):
            wt = singles.tile([128, KC, TCHUNK], f32r, tag=f"wt{j}")
            nc.sync.dma_start(out=wt,
                              in_=wqkv[j].rearrange("p (c n) -> p c n", c=KC))
            wts.append(wt)

        bg1t = bgwt[:, 0:1]
        wg2t = bgwt[:, 1:2]

        # --- gate phase: g[t] = aw*sigmoid(MLP(my)[t]) + (1-aw) ---
        gcol_all = singles.tile([128, NT], f32)
        for g in range(NG):
            sl = slice(g * TCHUNK, (g + 1) * TCHUNK)
            hps = ph.tile([128, TCHUNK], f32, tag="hps")
            for c in range(KC):
                nc.tensor.matmul(
                    hps, lhsT=w1t[:, c, :], rhs=mts[c][:, sl],
                    start=(c == 0), stop=(c == KC - 1),
                )
            hact = pact.tile([128, TCHUNK], f32, tag="hact")
            nc.scalar.activation(out=hact, in_=hps, func=GELU, bias=bg1t)
            for mi in range(MPG):
                m = g * MPG + mi
                glps = pgl.tile([128, 1], f32, tag="glps")
                nc.tensor.matmul(
                    glps, lhsT=hact[:, mi * 128:(mi + 1) * 128], rhs=wg2t,
                    start=True, stop=True,
                )
                gc = gcol_all[:, m:m + 1]
                nc.scalar.activation(out=gc, in_=glps, func=SIGM, bias=b2)
                nc.scalar.activation(out=gc, in_=gc, func=COPY,
                                     bias=1.0 - aw, scale=aw)

        # --- fused penta projections first (tiny, frees the tail) ---
        op_all = singles.tile([128, NT, PCOLS], f32)
        for m in range(NT):
            tsl = slice(m * 128, (m + 1) * 128)
            psp = pps.tile([128, PCOLS], f32, tag="psp")
            for c in range(KC):
                nc.tensor.matmul(
                    psp, lhsT=mts[c][:, tsl], rhs=wpt[:, c, :],
                    start=(c == 0), stop=(c == KC - 1),
                )
            nc.vector.tensor_scalar_mul(out=op_all[:, m, :], in0=psp,
                                        scalar1=gcol_all[:, m:m + 1])
        nc.sync.dma_start(out=p_out.rearrange("(m p) j -> p m j", p=128),
                          in_=op_all)

        # --- QKV projections, gating folded into eviction scale ---
        ev = 0
        for m in range(NT):
            tsl = slice(m * 128, (m + 1) * 128)
            gc = gcol_all[:, m:m + 1]
            for pi, out_dram in enumerate((q_out, k_out, v_out)):
                o = po.tile([128, EDIM], f32, tag="o")
                for nb in range(NB):
                    nsl = slice(nb * TCHUNK, (nb + 1) * TCHUNK)
                    wt = wts[pi * NB + nb]
                    ps = pq.tile([128, TCHUNK], f32, tag="ps")
                    for c in range(KC):
                        nc.tensor.matmul(
                            ps, lhsT=mts[c][:, tsl], rhs=wt[:, c, :],
                            start=(c == 0), stop=(c == KC - 1),
                        )
                    if ev % 3 == 2:
                        nc.scalar.activation(out=o[:, nsl], in_=ps, func=COPY,
                                             bias=0.0, scale=gc)
                    else:
                        nc.vector.tensor_scalar_mul(out=o[:, nsl], in0=ps,
                                                    scalar1=gc)
                    ev += 1
                nc.sync.dma_start(out=out_dram[tsl, :], in_=o)


def _prep_inputs(tokens, fingerprints, W_g1, b_g1, W_g2, b_g2, alpha, Wq, Wk,
                 Wv, pentachoron):
    """Host-side routing, gather, padding, weight prep. Returns
    (in_maps, T, n_tok, B, Psel, aw, b2) or None if no token selected."""
    f32 = np.float32
    tokens = np.asarray(tokens)
    fingerprints = np.asarray(fingerprints)
    B = tokens.shape[0]

    mask = (fingerprints >= FP_MIN) & (fingerprints < FP_MAX)
    idx = np.nonzero(mask)[0]
    Psel = int(idx.shape[0])
    n_tok = B * Psel
    if n_tok == 0:
        return None

    my = tokens[:, idx, SL_START:SL_END].astype(f32)  # [B, Psel, 512]
    flat = my.reshape(n_tok, SLICE)
    T = -(-n_tok // (N_CORES * TCHUNK)) * TCHUNK
    flatT = np.zeros((SLICE, N_CORES * T), f32)
    flatT[:, :n_tok] = flat.T

    W_g1 = np.asarray(W_g1, f32)
    W_g2 = np.asarray(W_g2, f32)
    b_g1 = np.asarray(b_g1, f32)
    b_g2 = np.asarray(b_g2, f32)
    alpha32 = np.asarray(alpha, f32)
    Wq = np.asarray(Wq, f32)
    Wk = np.asarray(Wk, f32)
    Wv = np.asarray(Wv, f32)
    penta = np.asarray(pentachoron, f32)

    aw = float(1.0 / (1.0 + np.exp(-alpha32)))
    b2 = float(b_g2.reshape(-1)[0])
    dirs = penta / np.linalg.norm(penta, axis=-1, keepdims=True)  # [5, EDIM]
    wp = np.concatenate(
        [
            (W.T.astype(np.float64) @ dirs.T.astype(np.float64)).astype(f32)
            for W in (Wq, Wk, Wv)
        ],
        axis=1,
    )  # [512, 15]
    wp = np.concatenate([wp, np.zeros((SLICE, PCOLS - 15), f32)], axis=1)

    bgw = np.stack([b_g1, W_g2.reshape(-1)], axis=1)  # [128, 2]

    in_common = {
        "wg1": _swz(np.ascontiguousarray(W_g1.T)),
        "bgw": np.ascontiguousarray(bgw),
        "wp": _swz(wp),
    }
    for nm, W in (("q", Wq), ("k", Wk), ("v", Wv)):
        WT = np.ascontiguousarray(W.T)  # [512, 1024]
        for i in range(NB):
            in_common[f"w{nm}{i}"] = _swz(WT[:, i * TCHUNK:(i + 1) * TCHUNK])

    in_maps = [
        {"myT": _swz(flatT[:, c * T:(c + 1) * T]), **in_common}
        for c in range(N_CORES)
    ]
    return in_maps, T, n_tok, B, Psel, aw, b2


def _build(T, aw, b2):
    import concourse.mybir as mybir
    import concourse.tile as tile
    from concourse import bacc

    nc = bacc.Bacc("TRN2", target_bir_lowering=False, debug=False,
                   num_devices=N_CORES)
    with tile.TileContext(nc) as tc:
        _trace_kernel(nc, tc, tile, mybir, T, aw, b2)
    nc.compile()
    return nc


def _unshard(res, T, n_tok, B, Psel):
    q = np.concatenate([res.results[c]["q"] for c in range(N_CORES)], axis=0)
    k = np.concatenate([res.results[c]["k"] for c in range(N_CORES)], axis=0)
    v = np.concatenate([res.results[c]["v"] for c in range(N_CORES)], axis=0)
    p = np.concatenate([res.results[c]["p"] for c in range(N_CORES)], axis=0)

    Q = q[:n_tok].reshape(B, Psel, EDIM)
    K = k[:n_tok].reshape(B, Psel, EDIM)
    V = v[:n_tok].reshape(B, Psel, EDIM)
    p = p[:n_tok]  # [n_tok, PCOLS]
    Qp = np.ascontiguousarray(p[:, 0:5].T).reshape(5, B, Psel)
    Kp = np.ascontiguousarray(p[:, 5:10].T).reshape(5, B, Psel)
    Vp = np.ascontiguousarray(p[:, 10:15].T).reshape(5, B, Psel)
    return (Q, K, V, Qp, Kp, Vp)


def kernel(tokens, fingerprints, W_g1, b_g1, W_g2, b_g2, alpha, Wq, Wk, Wv,
           pentachoron):
    from concourse.bass_utils import run_bass_kernel_spmd

    prep = _prep_inputs(tokens, fingerprints, W_g1, b_g1, W_g2, b_g2, alpha,
                        Wq, Wk, Wv, pentachoron)
    if prep is None:
        B = np.asarray(tokens).shape[0]
        z = np.zeros((B, 0, EDIM), np.float32)
        zp = np.zeros((5, B, 0), np.float32)
        return (z, z.copy(), z.copy(), zp, zp.copy(), zp.copy())
    in_maps, T, n_tok, B, Psel, aw, b2 = prep

    nc = _build(T, aw, b2)
    res = run_bass_kernel_spmd(nc, in_maps, list(range(N_CORES)))
    return _unshard(res, T, n_tok, B, Psel)
